# revision 5
# baseline (speedup 1.0000x reference)
"""Trainium2 Bass kernel for nn_CrossAttention (B=4, LQ=4096, S=4096, D=512).

Sharding: data-parallel over (batch, query-half): core = 2*b + half.
Each core computes cross-attention for one batch element and 2048 query rows.
K/V projections are recomputed by both cores of a pair.

Host-side marshalling pre-casts all matmul operands to bf16 and pre-transposes
query/target into the [feature, token] layout the PE needs (bit-identical to
the on-device DVE cast + PE transpose it replaces, but free on the device).
All heavy matmuls run bf16 with fp32 PSUM accumulation. Softmax skips the
max-subtraction (scores bounded ~ +-30, exp stays in fp32 range) and the
normalization is deferred to the output-projection epilogue via per-row
reciprocals. The score->exp->context chain is software-pipelined one s-chunk
deep so the PE never waits on the Exp activation, and each query block's
epilogue (row-sum collapse, output projection, store) is drained inside the
next block's score loop. The final query block is tapered (384/128) so the
last epilogue has little work after the last context matmul.
"""

import numpy as np
import ml_dtypes

B, LQ, S = 4, 4096, 4096
D = 512          # SRC == TGT == 512
P = 128
LQH = LQ // 2    # 2048 query rows per core
DC = D // P      # 4 chunks of the feature dims
SC = S // P      # 32 s-chunks of 128
IB = 512         # kv block width for the K/V projection loop
KB = S // IB     # 8 kv blocks
WS = [512, 512, 512, 384, 128]   # stage C query-block widths (sum = LQH)

_CACHED = {}


def _build_program():
    import concourse.bass as bass
    import concourse.mybir as mybir
    import concourse.tile as tile
    from concourse import bacc
    from concourse.masks import make_identity
    from contextlib import ExitStack

    f32 = mybir.dt.float32
    bf16 = mybir.dt.bfloat16
    AF = mybir.ActivationFunctionType
    OP = mybir.AluOpType

    nc = bacc.Bacc("TRN2", target_bir_lowering=False, debug=False, num_devices=8)

    qT_d = nc.dram_tensor("qT", [D, LQH], bf16, kind="ExternalInput").ap()
    tgtT_d = nc.dram_tensor("tgtT", [D, S], bf16, kind="ExternalInput").ap()
    w_d = {}
    b_d = {}
    for nm in ("wq", "wk", "wv", "wo"):
        w_d[nm] = nc.dram_tensor(nm, [D, D], bf16, kind="ExternalInput").ap()
    for nm in ("bq", "bk", "bv", "bo"):
        b_d[nm] = nc.dram_tensor(nm, [D], f32, kind="ExternalInput").ap()
    out_dram = nc.dram_tensor("out", [LQH, D], f32, kind="ExternalOutput").ap()

    with tile.TileContext(nc) as tc, ExitStack() as ctx:
        const = ctx.enter_context(tc.tile_pool(name="const", bufs=1))
        big = ctx.enter_context(tc.tile_pool(name="big", bufs=1))
        ptp = ctx.enter_context(tc.tile_pool(name="ptp", bufs=6))
        ctxp = ctx.enter_context(tc.tile_pool(name="ctxp", bufs=2))
        outp = ctx.enter_context(tc.tile_pool(name="outp", bufs=4))
        smallp = ctx.enter_context(tc.tile_pool(name="smallp", bufs=2))
        ps_pt = ctx.enter_context(tc.tile_pool(name="ps_pt", bufs=4, space="PSUM"))
        ps_ctx = ctx.enter_context(tc.tile_pool(name="ps_ctx", bufs=4, space="PSUM"))

        # ---- tiny constants first (identity keeps the PE busy at t=0) ----
        ident_f = const.tile([P, P], f32, tag="ident_f", name="ident_f")
        make_identity(nc, ident_f)
        ones_col = const.tile([P, 1], f32, tag="ones_col", name="ones_col")
        nc.vector.memset(ones_col, 1.0)

        b_col = {}
        for nm in ("bq", "bk"):
            bc = const.tile([P, DC], f32, tag=f"col_{nm}", name=f"{nm}_col")
            nc.gpsimd.dma_start(out=bc, in_=b_d[nm].rearrange("(c p) -> p c", p=P))
            b_col[nm] = bc
        b_rep = {}
        for nm in ("bv", "bo"):
            br = const.tile([P, D], f32, tag=f"rep_{nm}", name=f"{nm}_rep")
            src = b_d[nm]
            bcast = bass.AP(tensor=src.tensor, offset=src.offset,
                            ap=[[0, P]] + list(src.ap))
            nc.gpsimd.dma_start(out=br, in_=bcast)
            b_rep[nm] = br

        # staging row for the reciprocal transpose: row 0 live, rows 1.. zero
        rstage = const.tile([P, IB], f32, tag="rstage", name="rstage")
        nc.vector.memset(rstage, 0.0)

        # ---- big DMAs, spread across the three DMA-capable engine queues
        # (sync/scalar/gpsimd) so they run in parallel; a single queue
        # serializes and the first matmul waits ~8us extra ----
        w_t = {}
        engs = [nc.sync, nc.scalar, nc.gpsimd]

        def load_weight(nm, eng):
            wt = const.tile([P, DC, D], bf16, tag=f"w_{nm}", name=f"{nm}_bf")
            eng.dma_start(wt, w_d[nm].rearrange("(c p) n -> p c n", p=P))
            w_t[nm] = wt

        load_weight("wq", nc.sync)
        qinT = big.tile([P, DC, LQH], bf16, tag="qinT", name="qinT")
        qT_r = qT_d.rearrange("(c p) i -> p c i", p=P)
        for ibk in range(4):
            sl = slice(ibk * IB, (ibk + 1) * IB)
            engs[(1 + ibk) % 3].dma_start(qinT[:, :, sl], qT_r[:, :, sl])
        load_weight("wk", nc.gpsimd)
        load_weight("wv", nc.sync)
        tgtT = big.tile([P, DC, S], bf16, tag="tgtT", name="tgtT")
        tgtT_r = tgtT_d.rearrange("(c p) s -> p c s", p=P)
        for sb in range(KB):
            sl = slice(sb * IB, (sb + 1) * IB)
            engs[(1 + sb) % 3].dma_start(tgtT[:, :, sl], tgtT_r[:, :, sl])
        load_weight("wo", nc.scalar)

        # ---- stage A: query projection -> qTp[ib] = [dout, i] per C block ----
        offs = [sum(WS[:i]) for i in range(len(WS))]
        qTp = [big.tile([P, DC, W], bf16, tag=f"qTp{i}", name=f"qTp{i}")
               for i, W in enumerate(WS)]
        for ib, W in enumerate(WS):
            off = offs[ib]
            for tcc in range(DC):
                ps = ps_pt.tile([P, W], f32, tag="ps_pt", name=f"psq_{ib}_{tcc}")
                for dc in range(DC):
                    nc.tensor.matmul(ps, w_t["wq"][:, dc, tcc * P:(tcc + 1) * P],
                                     qinT[:, dc, off:off + W],
                                     start=(dc == 0), stop=(dc == DC - 1))
                nc.vector.tensor_tensor(qTp[ib][:, tcc, :], ps,
                                        b_col["bq"][:, tcc:tcc + 1].to_broadcast([P, W]),
                                        OP.add)

        # ---- stage B: K/V projections ----
        kT = big.tile([P, DC, S], bf16, tag="kT", name="kT")
        vv = big.tile([P, SC, D], bf16, tag="vv", name="vv")
        for sb in range(KB):
            ssl = slice(sb * IB, (sb + 1) * IB)
            for tcc in range(DC):
                ps = ps_pt.tile([P, IB], f32, tag="ps_pt", name=f"psk_{sb}_{tcc}")
                for dc in range(DC):
                    nc.tensor.matmul(ps, w_t["wk"][:, dc, tcc * P:(tcc + 1) * P],
                                     tgtT[:, dc, ssl],
                                     start=(dc == 0), stop=(dc == DC - 1))
                nc.vector.tensor_tensor(kT[:, tcc, ssl], ps,
                                        b_col["bk"][:, tcc:tcc + 1].to_broadcast([P, IB]),
                                        OP.add)
            for sl in range(IB // P):
                scc = sb * (IB // P) + sl
                ps = ps_pt.tile([P, D], f32, tag="ps_pt", name=f"psv_{sb}_{sl}")
                for dc in range(DC):
                    nc.tensor.matmul(ps, tgtT[:, dc, scc * P:(scc + 1) * P],
                                     w_t["wv"][:, dc, :],
                                     start=(dc == 0), stop=(dc == DC - 1))
                nc.vector.tensor_tensor(vv[:, scc, :], ps, b_rep["bv"], OP.add)

        # ---- stage C: attention + output projection ----
        def make_epilogue(ib, W, off, ctxT, acc):
            """Yields epilogue steps for block ib; drained inside the next
            block's score loop so the PE work here hides in its stream."""
            NCH = W // P
            rs_ps = ps_pt.tile([1, W], f32, tag="ps_pt", name=f"rs_{ib}")
            nc.tensor.matmul(rs_ps, ones_col, acc, start=True, stop=True)
            nc.vector.tensor_copy(out=rstage[0:1, :W], in_=rs_ps)
            yield
            rt_ps = ps_pt.tile([P, W], f32, tag="ps_pt", name=f"rt_{ib}")
            for ic in range(NCH):
                nc.tensor.transpose(rt_ps[:, ic * P:(ic + 1) * P],
                                    rstage[:, ic * P:(ic + 1) * P], ident_f)
            rsum_col = smallp.tile([P, NCH], f32, tag="rsum_col", name=f"rsc_{ib}")
            nc.scalar.activation(rsum_col,
                                 rt_ps.rearrange("p (c q) -> p c q", c=NCH)[:, :, 0],
                                 AF.Copy)
            rc_sb = smallp.tile([P, NCH], f32, tag="rc_sb", name=f"rc_{ib}")
            nc.vector.reciprocal(rc_sb, rsum_col)
            yield
            for ic in range(NCH):
                op_ps = ps_pt.tile([P, D], f32, tag="ps_pt", name=f"op_{ib}_{ic}")
                for dpc in range(DC):
                    nc.tensor.matmul(op_ps, ctxT[:, dpc, ic * P:(ic + 1) * P],
                                     w_t["wo"][:, dpc, :],
                                     start=(dpc == 0), stop=(dpc == DC - 1))
                ot_s = outp.tile([P, D], f32, tag="out_s", name=f"ots_{ib}_{ic}")
                nc.scalar.activation(ot_s, op_ps, AF.Copy,
                                     scale=rc_sb[:, ic:ic + 1])
                ot = outp.tile([P, D], f32, tag="out_t", name=f"ot_{ib}_{ic}")
                nc.vector.tensor_tensor(ot, ot_s, b_rep["bo"], OP.add)
                nc.sync.dma_start(out_dram[off + ic * P: off + (ic + 1) * P, :], ot)
                yield

        prev_epi = None
        for ib, W in enumerate(WS):
            off = offs[ib]
            ctx_ps = [ps_ctx.tile([P, W], f32, tag="ps_ctx", name=f"ctx_{ib}_{d}")
                      for d in range(DC)]
            acc = smallp.tile([P, W], f32, tag="rs_acc", name=f"rsacc_{ib}")

            def emit_ctx(pexp, pscc):
                for dpc in range(DC):
                    nc.tensor.matmul(ctx_ps[dpc], vv[:, pscc, dpc * P:(dpc + 1) * P],
                                     pexp, start=(pscc == 0), stop=(pscc == SC - 1))

            pend = None
            for scc in range(SC):
                pt_ps = ps_pt.tile([P, W], f32, tag="ps_pt", name=f"pt_{ib}_{scc}")
                for tcc in range(DC):
                    nc.tensor.matmul(pt_ps, kT[:, tcc, scc * P:(scc + 1) * P],
                                     qTp[ib][:, tcc, :],
                                     start=(tcc == 0), stop=(tcc == DC - 1))
                pt_exp = ptp.tile([P, W], bf16, tag="pt_exp", name=f"pte_{ib}_{scc}")
                nc.scalar.activation(pt_exp, pt_ps, AF.Exp)
                if scc == 0:
                    nc.vector.tensor_copy(out=acc, in_=pt_exp)
                else:
                    nc.vector.tensor_tensor(acc, acc, pt_exp, OP.add)
                if pend is not None:
                    emit_ctx(*pend)
                pend = (pt_exp, scc)
                if prev_epi is not None and 1 <= scc <= 10:
                    next(prev_epi, None)
            emit_ctx(*pend)

            # unnormalized context -> bf16 right away (frees the ctx banks)
            ctxT = ctxp.tile([P, DC, W], bf16, tag="ctxT", name=f"ctxT_{ib}")
            for dpc in range(DC):
                nc.vector.tensor_copy(out=ctxT[:, dpc, :], in_=ctx_ps[dpc])

            prev_epi = make_epilogue(ib, W, off, ctxT, acc)

        # drain the last block's epilogue
        for _ in prev_epi:
            pass

    nc.compile()
    return nc


def _get_nc():
    if "nc" not in _CACHED:
        _CACHED["nc"] = _build_program()
    return _CACHED["nc"]


def _make_in_maps(query, target, wq, bq, wk, bk, wv, bv, wo, bo):
    bf = ml_dtypes.bfloat16
    query = np.asarray(query, dtype=np.float32)
    target = np.asarray(target, dtype=np.float32)
    consts = {
        "wq": np.asarray(wq, np.float32).astype(bf),
        "wk": np.asarray(wk, np.float32).astype(bf),
        "wv": np.asarray(wv, np.float32).astype(bf),
        "wo": np.asarray(wo, np.float32).astype(bf),
        "bq": np.asarray(bq, np.float32), "bk": np.asarray(bk, np.float32),
        "bv": np.asarray(bv, np.float32), "bo": np.asarray(bo, np.float32),
    }
    in_maps = []
    for core in range(8):
        b, h = divmod(core, 2)
        qh = query[b, h * LQH:(h + 1) * LQH].astype(bf)     # [LQH, D]
        # faithful to the torch reshape: raw reinterpret of [512, 4096]
        tgt = np.ascontiguousarray(target[b]).reshape(S, D).astype(bf)
        in_maps.append({
            "qT": np.ascontiguousarray(qh.T),               # [D, LQH]
            "tgtT": np.ascontiguousarray(tgt.T),            # [D, S]
            **consts,
        })
    return in_maps


def kernel(query, target, wq, bq, wk, bk, wv, bv, wo, bo):
    from concourse import bass_utils
    nc = _get_nc()
    in_maps = _make_in_maps(query, target, wq, bq, wk, bk, wv, bv, wo, bo)
    res = bass_utils.run_bass_kernel_spmd(nc, in_maps, core_ids=list(range(8)))
    out = np.empty((B, LQ, D), np.float32)
    for core in range(8):
        b, h = divmod(core, 2)
        out[b, h * LQH:(h + 1) * LQH] = res.results[core]["out"]
    return out


# revision 10
# speedup vs baseline: 1.0107x; 1.0107x over previous
"""Trainium2 Bass kernel for nn_CrossAttention (B=4, LQ=4096, S=4096, D=512).

Sharding: data-parallel over (batch, query-half): core = 2*b + half.
Each core computes cross-attention for one batch element and 2048 query rows.
K/V projections are recomputed by both cores of a pair.

Host-side marshalling pre-casts all matmul operands to bf16 and pre-transposes
query/target into the [feature, token] layout the PE needs (bit-identical to
the on-device DVE cast + PE transpose it replaces, but free on the device).
All heavy matmuls run bf16 with fp32 PSUM accumulation. Softmax skips the
max-subtraction (scores bounded ~ +-30, exp stays in fp32 range) and the
normalization is deferred to the output-projection epilogue via per-row
reciprocals. The score->exp->context chain is software-pipelined one s-chunk
deep so the PE never waits on the Exp activation, and each query block's
epilogue (row-sum collapse, output projection, store) is drained inside the
next block's score loop. The final query block is tapered (384/128) so the
last epilogue has little work after the last context matmul.
"""

import numpy as np
import ml_dtypes

B, LQ, S = 4, 4096, 4096
D = 512          # SRC == TGT == 512
P = 128
LQH = LQ // 2    # 2048 query rows per core
DC = D // P      # 4 chunks of the feature dims
SC = S // P      # 32 s-chunks of 128
IB = 512         # kv block width for the K/V projection loop
KB = S // IB     # 8 kv blocks
WS = [512, 512, 512, 384, 128]   # stage C query-block widths (sum = LQH)

_CACHED = {}


def _build_program():
    import concourse.bass as bass
    import concourse.mybir as mybir
    import concourse.tile as tile
    from concourse import bacc
    from concourse.masks import make_identity
    from contextlib import ExitStack

    f32 = mybir.dt.float32
    bf16 = mybir.dt.bfloat16
    AF = mybir.ActivationFunctionType
    OP = mybir.AluOpType

    nc = bacc.Bacc("TRN2", target_bir_lowering=False, debug=False, num_devices=8)

    qT_d = nc.dram_tensor("qT", [D, LQH], bf16, kind="ExternalInput").ap()
    tgtT_d = nc.dram_tensor("tgtT", [D, S], bf16, kind="ExternalInput").ap()
    w_d = {}
    b_d = {}
    for nm in ("wq", "wk", "wv", "wo"):
        w_d[nm] = nc.dram_tensor(nm, [D, D], bf16, kind="ExternalInput").ap()
    for nm in ("bq", "bk", "bv", "bo"):
        b_d[nm] = nc.dram_tensor(nm, [D], f32, kind="ExternalInput").ap()
    out_dram = nc.dram_tensor("out", [LQH, D], f32, kind="ExternalOutput").ap()

    with tile.TileContext(nc) as tc, ExitStack() as ctx:
        const = ctx.enter_context(tc.tile_pool(name="const", bufs=1))
        big = ctx.enter_context(tc.tile_pool(name="big", bufs=1))
        ptp = ctx.enter_context(tc.tile_pool(name="ptp", bufs=6))
        ctxp = ctx.enter_context(tc.tile_pool(name="ctxp", bufs=2))
        outp = ctx.enter_context(tc.tile_pool(name="outp", bufs=4))
        smallp = ctx.enter_context(tc.tile_pool(name="smallp", bufs=2))
        ps_pt = ctx.enter_context(tc.tile_pool(name="ps_pt", bufs=4, space="PSUM"))
        ps_ctx = ctx.enter_context(tc.tile_pool(name="ps_ctx", bufs=4, space="PSUM"))

        # ---- tiny constants first (identity keeps the PE busy at t=0) ----
        ident_f = const.tile([P, P], f32, tag="ident_f", name="ident_f")
        make_identity(nc, ident_f)
        ones_col = const.tile([P, 1], f32, tag="ones_col", name="ones_col")
        nc.vector.memset(ones_col, 1.0)

        b_col = {}
        for nm in ("bq", "bk"):
            bc = const.tile([P, DC], f32, tag=f"col_{nm}", name=f"{nm}_col")
            nc.gpsimd.dma_start(out=bc, in_=b_d[nm].rearrange("(c p) -> p c", p=P))
            b_col[nm] = bc
        b_rep = {}
        for nm in ("bv", "bo"):
            br = const.tile([P, D], f32, tag=f"rep_{nm}", name=f"{nm}_rep")
            src = b_d[nm]
            bcast = bass.AP(tensor=src.tensor, offset=src.offset,
                            ap=[[0, P]] + list(src.ap))
            nc.gpsimd.dma_start(out=br, in_=bcast)
            b_rep[nm] = br

        # staging row for the reciprocal transpose: row 0 live, rows 1.. zero
        rstage = const.tile([P, IB], f32, tag="rstage", name="rstage")
        nc.vector.memset(rstage, 0.0)

        # ---- big DMAs: one fast (sync HWDGE) queue, strictly in order of
        # first use, with the first-needed tensors split fine-grained so the
        # PE starts as early as possible ----
        w_t = {}

        def load_weight(nm, chunks=1):
            wt = const.tile([P, DC, D], bf16, tag=f"w_{nm}", name=f"{nm}_bf")
            src = w_d[nm].rearrange("(c p) n -> p c n", p=P)
            cw = D // chunks
            for c in range(chunks):
                nc.sync.dma_start(wt[:, :, c * cw:(c + 1) * cw],
                                  src[:, :, c * cw:(c + 1) * cw])
            w_t[nm] = wt

        QG = 256                       # query DMA/projection granule
        NQG = LQH // QG
        qinT = big.tile([P, DC, LQH], bf16, tag="qinT", name="qinT")
        qT_r = qT_d.rearrange("(c p) i -> p c i", p=P)
        tgtT = big.tile([P, DC, S], bf16, tag="tgtT", name="tgtT")
        tgtT_r = tgtT_d.rearrange("(c p) s -> p c s", p=P)

        def load_qin(g):
            sl = slice(g * QG, (g + 1) * QG)
            nc.sync.dma_start(qinT[:, :, sl], qT_r[:, :, sl])

        def load_tgt(sb):
            sl = slice(sb * IB, (sb + 1) * IB)
            nc.sync.dma_start(tgtT[:, :, sl], tgtT_r[:, :, sl])

        load_weight("wq", chunks=4)
        load_qin(0)
        load_qin(1)
        load_weight("wk")
        for g in range(2, NQG):
            load_qin(g)
        load_tgt(0)
        load_tgt(1)
        load_weight("wv")
        for sb in range(2, KB):
            load_tgt(sb)
        load_weight("wo")

        # ---- stage A: query projection -> qTp = [dout, i] ----
        qTp = big.tile([P, DC, LQH], bf16, tag="qTp", name="qTp")
        for g in range(NQG):
            for tcc in range(DC):
                ps = ps_pt.tile([P, QG], f32, tag="ps_pt", name=f"psq_{g}_{tcc}")
                for dc in range(DC):
                    nc.tensor.matmul(ps, w_t["wq"][:, dc, tcc * P:(tcc + 1) * P],
                                     qinT[:, dc, g * QG:(g + 1) * QG],
                                     start=(dc == 0), stop=(dc == DC - 1))
                nc.vector.tensor_tensor(qTp[:, tcc, g * QG:(g + 1) * QG], ps,
                                        b_col["bq"][:, tcc:tcc + 1].to_broadcast([P, QG]),
                                        OP.add)

        # ---- stage B: K/V projections ----
        kT = big.tile([P, DC, S], bf16, tag="kT", name="kT")
        vv = big.tile([P, SC, D], bf16, tag="vv", name="vv")
        for sb in range(KB):
            ssl = slice(sb * IB, (sb + 1) * IB)
            for tcc in range(DC):
                ps = ps_pt.tile([P, IB], f32, tag="ps_pt", name=f"psk_{sb}_{tcc}")
                for dc in range(DC):
                    nc.tensor.matmul(ps, w_t["wk"][:, dc, tcc * P:(tcc + 1) * P],
                                     tgtT[:, dc, ssl],
                                     start=(dc == 0), stop=(dc == DC - 1))
                nc.vector.tensor_tensor(kT[:, tcc, ssl], ps,
                                        b_col["bk"][:, tcc:tcc + 1].to_broadcast([P, IB]),
                                        OP.add)
            for sl in range(IB // P):
                scc = sb * (IB // P) + sl
                ps = ps_pt.tile([P, D], f32, tag="ps_pt", name=f"psv_{sb}_{sl}")
                for dc in range(DC):
                    nc.tensor.matmul(ps, tgtT[:, dc, scc * P:(scc + 1) * P],
                                     w_t["wv"][:, dc, :],
                                     start=(dc == 0), stop=(dc == DC - 1))
                nc.vector.tensor_tensor(vv[:, scc, :], ps, b_rep["bv"], OP.add)

        # ---- stage C: attention + output projection ----
        def make_epilogue(ib, W, off, ctxT, acc):
            """Yields epilogue steps for block ib; drained inside the next
            block's score loop so the PE work here hides in its stream."""
            NCH = W // P
            rs_ps = ps_pt.tile([1, W], f32, tag="ps_pt", name=f"rs_{ib}")
            nc.tensor.matmul(rs_ps, ones_col, acc, start=True, stop=True)
            nc.vector.tensor_copy(out=rstage[0:1, :W], in_=rs_ps)
            yield
            rt_ps = ps_pt.tile([P, W], f32, tag="ps_pt", name=f"rt_{ib}")
            for ic in range(NCH):
                nc.tensor.transpose(rt_ps[:, ic * P:(ic + 1) * P],
                                    rstage[:, ic * P:(ic + 1) * P], ident_f)
            rsum_col = smallp.tile([P, NCH], f32, tag="rsum_col", name=f"rsc_{ib}")
            nc.scalar.activation(rsum_col,
                                 rt_ps.rearrange("p (c q) -> p c q", c=NCH)[:, :, 0],
                                 AF.Copy)
            rc_sb = smallp.tile([P, NCH], f32, tag="rc_sb", name=f"rc_{ib}")
            nc.vector.reciprocal(rc_sb, rsum_col)
            yield
            for ic in range(NCH):
                op_ps = ps_pt.tile([P, D], f32, tag="ps_pt", name=f"op_{ib}_{ic}")
                for dpc in range(DC):
                    nc.tensor.matmul(op_ps, ctxT[:, dpc, ic * P:(ic + 1) * P],
                                     w_t["wo"][:, dpc, :],
                                     start=(dpc == 0), stop=(dpc == DC - 1))
                ot_s = outp.tile([P, D], f32, tag="out_s", name=f"ots_{ib}_{ic}")
                nc.scalar.activation(ot_s, op_ps, AF.Copy,
                                     scale=rc_sb[:, ic:ic + 1])
                ot = outp.tile([P, D], f32, tag="out_t", name=f"ot_{ib}_{ic}")
                nc.vector.tensor_tensor(ot, ot_s, b_rep["bo"], OP.add)
                nc.sync.dma_start(out_dram[off + ic * P: off + (ic + 1) * P, :], ot)
                yield

        offs = [sum(WS[:i]) for i in range(len(WS))]
        prev_epi = None
        for ib, W in enumerate(WS):
            off = offs[ib]
            ctx_ps = [ps_ctx.tile([P, W], f32, tag="ps_ctx", name=f"ctx_{ib}_{d}")
                      for d in range(DC)]
            acc = smallp.tile([P, W], f32, tag="rs_acc", name=f"rsacc_{ib}")

            def emit_ctx(pexp, pscc):
                for dpc in range(DC):
                    nc.tensor.matmul(ctx_ps[dpc], vv[:, pscc, dpc * P:(dpc + 1) * P],
                                     pexp, start=(pscc == 0), stop=(pscc == SC - 1))

            pend = None
            for scc in range(SC):
                pt_ps = ps_pt.tile([P, W], f32, tag="ps_pt", name=f"pt_{ib}_{scc}")
                for tcc in range(DC):
                    nc.tensor.matmul(pt_ps, kT[:, tcc, scc * P:(scc + 1) * P],
                                     qTp[:, tcc, off:off + W],
                                     start=(tcc == 0), stop=(tcc == DC - 1))
                pt_exp = ptp.tile([P, W], bf16, tag="pt_exp", name=f"pte_{ib}_{scc}")
                nc.scalar.activation(pt_exp, pt_ps, AF.Exp)
                if scc == 0:
                    nc.vector.tensor_copy(out=acc, in_=pt_exp)
                else:
                    nc.vector.tensor_tensor(acc, acc, pt_exp, OP.add)
                if pend is not None:
                    emit_ctx(*pend)
                pend = (pt_exp, scc)
                if prev_epi is not None and 1 <= scc <= 10:
                    next(prev_epi, None)
            emit_ctx(*pend)

            # unnormalized context -> bf16 right away (frees the ctx banks)
            ctxT = ctxp.tile([P, DC, W], bf16, tag="ctxT", name=f"ctxT_{ib}")
            for dpc in range(DC):
                nc.vector.tensor_copy(out=ctxT[:, dpc, :], in_=ctx_ps[dpc])

            prev_epi = make_epilogue(ib, W, off, ctxT, acc)

        # drain the last block's epilogue
        for _ in prev_epi:
            pass

    nc.compile()
    return nc


def _get_nc():
    if "nc" not in _CACHED:
        _CACHED["nc"] = _build_program()
    return _CACHED["nc"]


def _make_in_maps(query, target, wq, bq, wk, bk, wv, bv, wo, bo):
    bf = ml_dtypes.bfloat16
    query = np.asarray(query, dtype=np.float32)
    target = np.asarray(target, dtype=np.float32)
    consts = {
        "wq": np.asarray(wq, np.float32).astype(bf),
        "wk": np.asarray(wk, np.float32).astype(bf),
        "wv": np.asarray(wv, np.float32).astype(bf),
        "wo": np.asarray(wo, np.float32).astype(bf),
        "bq": np.asarray(bq, np.float32), "bk": np.asarray(bk, np.float32),
        "bv": np.asarray(bv, np.float32), "bo": np.asarray(bo, np.float32),
    }
    in_maps = []
    for core in range(8):
        b, h = divmod(core, 2)
        qh = query[b, h * LQH:(h + 1) * LQH].astype(bf)     # [LQH, D]
        # faithful to the torch reshape: raw reinterpret of [512, 4096]
        tgt = np.ascontiguousarray(target[b]).reshape(S, D).astype(bf)
        in_maps.append({
            "qT": np.ascontiguousarray(qh.T),               # [D, LQH]
            "tgtT": np.ascontiguousarray(tgt.T),            # [D, S]
            **consts,
        })
    return in_maps


def kernel(query, target, wq, bq, wk, bk, wv, bv, wo, bo):
    from concourse import bass_utils
    nc = _get_nc()
    in_maps = _make_in_maps(query, target, wq, bq, wk, bk, wv, bv, wo, bo)
    res = bass_utils.run_bass_kernel_spmd(nc, in_maps, core_ids=list(range(8)))
    out = np.empty((B, LQ, D), np.float32)
    for core in range(8):
        b, h = divmod(core, 2)
        out[b, h * LQH:(h + 1) * LQH] = res.results[core]["out"]
    return out
